# revision 1
# baseline (speedup 1.0000x reference)
"""Trainium2 Bass kernel for nn_ExpABPropagation (gnn_message_passing).

out = exp(-eta * A_dad @ diag(mask)) @ residuals, 40-term Taylor reference.

Math: with M = -A_dad @ diag(mask), term_k = M^k r / k!, out = sum_k term_k.
Only masked components of term feed the next term, so the recurrence lives on
the ~N/2 masked nodes.  Absorbing the D^{-1/2} factors into the state turns
each iteration into an UNWEIGHTED gather + segment-sum + per-row scale:

    zhat_k = g  *  (Cnt_mm @ zhat_{k-1}),   zhat_0 = dis * r   (masked rows)
    stilde  = sum_{k=1..K} alpha_k zhat_{k-1},  alpha_k = (-1)^k / k!
    out     = r + dis * (Cnt_am @ stilde)

where Cnt_* are 0/1(multiplicity) adjacency counts, g = 1/deg, dis = deg^-1/2.
The Taylor tail after K=16 terms is < 1/17! ~ 3e-15, far below f32 eps, so 15
on-device SpMMs replace the reference's 40.

Device mapping per iteration (SPMD on 8 NeuronCores):
  - nodes (tokens) sorted by degree, dealt block-of-128-wise to cores
  - per destination-row gather slots padded to a per-chunk uniform depth L;
    dma_gather (256B/token, int16 idx, lo/hi source split at 32768) pulls
    source rows from the AllGather'd full state in DRAM
  - DVE strided tensor_reduce sums each row's L slots (row == partition)
  - per-row scale via a host-precomputed broadcast tensor, AllGather shares
    the new state for the next iteration.
"""

import numpy as np

N_CORES = 8
PART = 128
C_FEAT = 64
LO_LIM = 32768
ELEM_BYTES = 256  # 64 * f32


# ----------------------------------------------------------------- planning


class _Plan:
    pass


def _aligned_slots(csr_indptr, csr_src, slot_node, n_blocks, chunk_blocks,
                   L_caps=None):
    """Per-(chunk,half) slot depth bookkeeping for one core.

    slot_node: [n_blocks*128] node id per slot (-1 pad).
    Returns per-slot lo/hi source-token lists (ragged) + per-chunk maxima.
    """
    n_slots = n_blocks * PART
    lo_lists = []
    hi_lists = []
    for s in range(n_slots):
        nd = slot_node[s]
        if nd < 0:
            lo_lists.append(np.empty(0, np.int64))
            hi_lists.append(np.empty(0, np.int64))
            continue
        srcs = csr_src[csr_indptr[nd]:csr_indptr[nd + 1]]
        lo_lists.append(srcs[srcs < LO_LIM])
        hi_lists.append(srcs[srcs >= LO_LIM] - LO_LIM)
    return lo_lists, hi_lists


def _chunk_ranges(n_blocks, chunk_blocks):
    out = []
    b = 0
    while b < n_blocks:
        nb = min(chunk_blocks, n_blocks - b)
        out.append((b, nb))
        b += nb
    return out


def _adaptive_chunks(blk_Llo, blk_Lhi, max_nb, g_budget):
    """Greedy chunking: grow a chunk while nb*max(L) stays within g_budget."""
    chunks, L_lo, L_hi = [], [], []
    b, n_blocks = 0, len(blk_Llo)
    while b < n_blocks:
        nb, mlo, mhi = 1, blk_Llo[b], blk_Lhi[b]
        assert mlo <= g_budget and mhi <= g_budget, (mlo, mhi, g_budget)
        while b + nb < n_blocks and nb < max_nb:
            nlo = max(mlo, blk_Llo[b + nb])
            nhi = max(mhi, blk_Lhi[b + nb])
            if (nb + 1) * nlo > g_budget or (nb + 1) * nhi > g_budget:
                break
            mlo, mhi, nb = nlo, nhi, nb + 1
        chunks.append((b, nb))
        L_lo.append(mlo)
        L_hi.append(mhi)
        b += nb
    return chunks, L_lo, L_hi


def _block_maxima(per_core_lists, n_blocks):
    """Per-local-block max slot depth across all cores."""
    out = np.zeros(n_blocks, np.int64)
    for lists in per_core_lists:
        lens = np.array([len(x) for x in lists], np.int64).reshape(n_blocks,
                                                                   PART)
        out = np.maximum(out, lens.max(axis=1))
    return out


def _build_idx_cols(lo_lists, hi_lists, chunks, L_lo, L_hi, zero_lo, zero_hi):
    """Build the packed int16 idx column array [128, total_cols] for one core.

    Stream order per chunk: lo groups (nb*L_lo groups of 128), then hi.
    idx position i -> (partition i%16 replicated 8x, column i//16).
    Returns (idx_cols [128, total_cols] int16, per-chunk (off_lo, off_hi)).
    """
    streams = []
    offs = []
    col_off = 0
    for ci, (b0, nb) in enumerate(chunks):
        llo, lhi = L_lo[ci], L_hi[ci]
        o_lo = col_off
        if llo > 0:
            arr = np.full((nb, llo, PART), zero_lo, np.int64)
            for bl in range(nb):
                for p in range(PART):
                    srcs = lo_lists[(b0 + bl) * PART + p]
                    arr[bl, :len(srcs), p] = srcs
            streams.append(arr.reshape(-1))
            col_off += nb * llo * PART // 16
        o_hi = col_off
        if lhi > 0:
            arr = np.full((nb, lhi, PART), zero_hi, np.int64)
            for bl in range(nb):
                for p in range(PART):
                    srcs = lhi_s = hi_lists[(b0 + bl) * PART + p]
                    arr[bl, :len(srcs), p] = srcs
            streams.append(arr.reshape(-1))
            col_off += nb * lhi * PART // 16
        offs.append((o_lo, o_hi))
    if streams:
        flat = np.concatenate(streams)
    else:
        flat = np.zeros(16, np.int64)
        col_off = 1
    assert flat.size % 16 == 0
    cols = flat.reshape(-1, 16).T  # [16, total_cols]
    idx = np.tile(cols, (8, 1)).astype(np.int16)  # replicate to 128 partitions
    return idx, offs, col_off


def _plan(residuals, edge_index, train_mask, num_nodes, K=16,
          g_budget_rec=100, g_budget_fin=80, max_nb=10):
    p = _Plan()
    N = int(num_nodes)
    row = np.asarray(edge_index[0], np.int64)
    col = np.asarray(edge_index[1], np.int64)
    mask = np.asarray(train_mask) != 0
    resid = np.asarray(residuals, np.float32)

    deg = np.bincount(row, minlength=N).astype(np.float64)
    dis = np.where(deg > 0, 1.0 / np.sqrt(np.maximum(deg, 1.0)), 0.0)
    g_node = dis * dis

    masked = np.nonzero(mask)[0]
    N_m = len(masked)
    assert N_m > 0

    # ---- token space: sort masked nodes by masked-masked in-degree --------
    em = mask[row] & mask[col]
    mm_row, mm_col = row[em], col[em]
    mm_indeg = np.bincount(mm_row, minlength=N)[masked]
    order = np.argsort(-mm_indeg, kind="stable")
    sorted_nodes = masked[order]

    # reserved all-zero tokens (gather padding targets), in lo + hi ranges
    B_r = int(np.ceil((N_m + 2) / (N_CORES * PART)))
    TPC = B_r * PART
    N_tok = N_CORES * TPC
    reserved = {0}
    if N_tok > LO_LIM:
        reserved.add(LO_LIM)
    zero_lo = 0
    zero_hi = 0  # relative to LO_LIM

    slot_node_rec = np.full(N_tok, -1, np.int64)  # token -> node
    tok_of_node = np.full(N, -1, np.int64)
    si = 0
    for gb in range(N_CORES * B_r):
        core, lb = gb % N_CORES, gb // N_CORES
        base = core * TPC + lb * PART
        for pp in range(PART):
            t = base + pp
            if t in reserved:
                continue
            if si < N_m:
                nd = sorted_nodes[si]
                slot_node_rec[t] = nd
                tok_of_node[nd] = t
                si += 1
    assert si == N_m

    # ---- recurrence CSR (dst node -> source tokens), masked-masked edges --
    src_tok = tok_of_node[mm_col]
    o2 = np.argsort(mm_row, kind="stable")
    r_sorted = mm_row[o2]
    s_sorted = src_tok[o2]
    indptr = np.zeros(N + 1, np.int64)
    np.add.at(indptr, r_sorted + 1, 1)
    np.cumsum(indptr, out=indptr)
    # randomize nothing; order within row irrelevant

    per_core_lo = []
    per_core_hi = []
    for c in range(N_CORES):
        sn = slot_node_rec[c * TPC:(c + 1) * TPC]
        lo_l, hi_l = _aligned_slots(indptr, s_sorted, sn, B_r, 0)
        per_core_lo.append(lo_l)
        per_core_hi.append(hi_l)

    blk_lo = _block_maxima(per_core_lo, B_r)
    blk_hi = _block_maxima(per_core_hi, B_r)
    chunks_rec, L_lo_rec, L_hi_rec = _adaptive_chunks(
        blk_lo, blk_hi, max_nb, g_budget_rec)

    idx_rec, offs_rec, cols_rec = [], [], 0
    for c in range(N_CORES):
        idx, offs, ncols = _build_idx_cols(per_core_lo[c], per_core_hi[c],
                                           chunks_rec, L_lo_rec, L_hi_rec,
                                           zero_lo, zero_hi)
        idx_rec.append(idx)
        offs_rec = offs
        cols_rec = ncols

    # ---- per-core row-scale broadcast + Z0 --------------------------------
    g_bcast = np.zeros((N_CORES, PART, B_r * C_FEAT), np.float32)
    for c in range(N_CORES):
        sn = slot_node_rec[c * TPC:(c + 1) * TPC]
        gv = np.where(sn >= 0, g_node[np.maximum(sn, 0)], 0.0)  # [TPC]
        g_bcast[c] = np.repeat(gv.reshape(B_r, PART).T[:, :, None],
                               C_FEAT, axis=2).reshape(PART, B_r * C_FEAT)

    Z0 = np.zeros((N_tok, C_FEAT), np.float32)
    valid = slot_node_rec >= 0
    Z0[valid] = (dis[slot_node_rec[valid]][:, None]
                 * resid[slot_node_rec[valid]]).astype(np.float32)

    # ---- final pass: all rows, col-masked edges ---------------------------
    R = int(np.ceil(N / N_CORES))
    B_f = int(np.ceil(R / PART))
    SLOTS_F = B_f * PART
    ec = mask[col]
    f_row, f_col = row[ec], col[ec]
    f_src = tok_of_node[f_col]
    assert (f_src >= 0).all()
    o3 = np.argsort(f_row, kind="stable")
    fr_sorted, fs_sorted = f_row[o3], f_src[o3]
    f_indptr = np.zeros(N + 1, np.int64)
    np.add.at(f_indptr, fr_sorted + 1, 1)
    np.cumsum(f_indptr, out=f_indptr)
    cm_indeg = np.diff(f_indptr)

    slot_node_fin = np.full((N_CORES, SLOTS_F), -1, np.int64)
    per_core_lo_f, per_core_hi_f = [], []
    for c in range(N_CORES):
        rows_c = np.arange(c * R, min((c + 1) * R, N))
        ordf = np.argsort(-cm_indeg[rows_c], kind="stable")
        rows_sorted = rows_c[ordf]
        slot_node_fin[c, :len(rows_sorted)] = rows_sorted
        lo_l, hi_l = _aligned_slots(f_indptr, fs_sorted, slot_node_fin[c],
                                    B_f, 0)
        per_core_lo_f.append(lo_l)
        per_core_hi_f.append(hi_l)

    blk_lo_f = _block_maxima(per_core_lo_f, B_f)
    blk_hi_f = _block_maxima(per_core_hi_f, B_f)
    chunks_fin, L_lo_fin, L_hi_fin = _adaptive_chunks(
        blk_lo_f, blk_hi_f, max_nb, g_budget_fin)

    idx_fin, offs_fin, cols_fin = [], [], 0
    for c in range(N_CORES):
        idx, offs, ncols = _build_idx_cols(per_core_lo_f[c], per_core_hi_f[c],
                                           chunks_fin, L_lo_fin, L_hi_fin,
                                           zero_lo, zero_hi)
        idx_fin.append(idx)
        offs_fin = offs
        cols_fin = ncols

    g2_bcast = np.zeros((N_CORES, PART, B_f * C_FEAT), np.float32)
    resid_perm = np.zeros((N_CORES, SLOTS_F, C_FEAT), np.float32)
    for c in range(N_CORES):
        sn = slot_node_fin[c]
        gv = np.where(sn >= 0, dis[np.maximum(sn, 0)], 0.0)
        g2_bcast[c] = np.repeat(gv.reshape(B_f, PART).T[:, :, None],
                                C_FEAT, axis=2).reshape(PART, B_f * C_FEAT)
        v = sn >= 0
        resid_perm[c][v] = resid[sn[v]]

    # Taylor coefficients alpha_k = (-1)^k / k!
    alpha = np.zeros(K + 1, np.float64)
    a = 1.0
    for k in range(1, K + 1):
        a = -a / k
        alpha[k] = a

    p.N, p.N_m, p.N_tok, p.TPC, p.B_r, p.B_f = N, N_m, N_tok, TPC, B_r, B_f
    p.SLOTS_F, p.R, p.K = SLOTS_F, R, K
    p.chunks_rec, p.L_lo_rec, p.L_hi_rec, p.offs_rec, p.cols_rec = (
        chunks_rec, L_lo_rec, L_hi_rec, offs_rec, cols_rec)
    p.chunks_fin, p.L_lo_fin, p.L_hi_fin, p.offs_fin, p.cols_fin = (
        chunks_fin, L_lo_fin, L_hi_fin, offs_fin, cols_fin)
    p.idx_rec, p.idx_fin = idx_rec, idx_fin
    p.g_bcast, p.g2_bcast, p.resid_perm = g_bcast, g2_bcast, resid_perm
    p.Z0, p.alpha, p.slot_node_fin = Z0, alpha, slot_node_fin
    return p


# ----------------------------------------------------------------- builder


def _build_kernel(nc, tc, p):
    from concourse import mybir
    f32 = mybir.dt.float32
    i16 = mybir.dt.int16
    AX = mybir.AxisListType
    OP = mybir.AluOpType
    B_r, B_f, TPC, N_tok, K = p.B_r, p.B_f, p.TPC, p.N_tok, p.K
    SLOTS_F = p.SLOTS_F
    n_lo_tok = min(N_tok, LO_LIM)
    n_hi_tok = N_tok - n_lo_tok

    z0_full = nc.dram_tensor("z0_full", [N_tok, C_FEAT], f32,
                             kind="ExternalInput")
    z0_shard = nc.dram_tensor("z0_shard", [TPC, C_FEAT], f32,
                              kind="ExternalInput")
    idxr_d = nc.dram_tensor("idx_rec", [PART, p.cols_rec], i16,
                            kind="ExternalInput")
    idxf_d = nc.dram_tensor("idx_fin", [PART, p.cols_fin], i16,
                            kind="ExternalInput")
    gb_d = nc.dram_tensor("g_bcast", [PART, B_r * C_FEAT], f32,
                          kind="ExternalInput")
    g2b_d = nc.dram_tensor("g2_bcast", [PART, B_f * C_FEAT], f32,
                           kind="ExternalInput")
    resid_d = nc.dram_tensor("resid_perm", [SLOTS_F, C_FEAT], f32,
                             kind="ExternalInput")
    out_d = nc.dram_tensor("out_shard", [SLOTS_F, C_FEAT], f32,
                           kind="ExternalOutput")

    rg = [list(range(N_CORES))]

    max_glo_r = max([nb * L for (b0, nb), L in zip(p.chunks_rec, p.L_lo_rec)]
                    + [1])
    max_ghi_r = max([nb * L for (b0, nb), L in zip(p.chunks_rec, p.L_hi_rec)]
                    + [1])
    max_glo_f = max([nb * L for (b0, nb), L in zip(p.chunks_fin, p.L_lo_fin)]
                    + [1])
    max_ghi_f = max([nb * L for (b0, nb), L in zip(p.chunks_fin, p.L_hi_fin)]
                    + [1])

    with tc.tile_pool(name="dram", bufs=1, space="DRAM") as dram:
        z_shard = dram.tile([TPC, C_FEAT], f32, tag="z_shard")
        z_fulls = [dram.tile([N_tok, C_FEAT], f32, tag=f"z_full{k}",
                             name=f"z_full{k}")
                   for k in range(1, K - 1)]
        st_shard = dram.tile([TPC, C_FEAT], f32, tag="st_shard")
        st_full = dram.tile([N_tok, C_FEAT], f32, tag="st_full",
                            name="st_full")

        with (
            tc.tile_pool(name="const", bufs=1) as constp,
            tc.tile_pool(name="msg", bufs=2) as msgp,
            tc.tile_pool(name="ytmp", bufs=3) as ytp,
        ):
            idx_t = constp.tile([PART, p.cols_rec], i16, tag="idx_rec")
            g_t = constp.tile([PART, B_r * C_FEAT], f32, tag="g_bcast")
            st_t = constp.tile([PART, B_r * C_FEAT], f32, tag="stilde")
            nc.sync.dma_start(idx_t[:], idxr_d[:])
            nc.sync.dma_start(g_t[:], gb_d[:])
            # stilde init: alpha_1 * zhat_0 = -1 * z0_shard
            nc.sync.dma_start(
                st_t[:].rearrange("p (b f) -> p b f", f=C_FEAT),
                z0_shard.ap().rearrange("(b q) f -> q b f", q=PART))
            nc.vector.tensor_scalar_mul(st_t[:], st_t[:], -1.0)

            def spmm_chunk(ci, b0, nb, llo, lhi, offs, idx_tile, src_lo, src_hi,
                           out_cb):
                """gather + reduce one chunk; returns zhat-free y tile
                (caller scales).  out_cb(y_ap, nb, b0) consumes the sum."""
                o_lo, o_hi = offs[ci]
                ylo = yhi = None
                if llo > 0:
                    glo = nb * llo
                    m = msgp.tile([PART, max_glo_r, C_FEAT], f32, tag="msg_lo")
                    n_idx = PART * glo
                    nc.gpsimd.dma_gather(
                        m[:, :glo, :], src_lo,
                        idx_tile[:, o_lo:o_lo + n_idx // 16],
                        n_idx, n_idx, C_FEAT,
                        single_packet=False)
                    ylo = ytp.tile([PART, nb * C_FEAT], f32, tag="ylo")
                    nc.vector.tensor_reduce(
                        ylo[:],
                        m[:, :glo, :].rearrange("p (b l) f -> p b f l", l=llo),
                        AX.X, OP.add)
                if lhi > 0:
                    ghi = nb * lhi
                    m = msgp.tile([PART, max_ghi_r, C_FEAT], f32, tag="msg_hi")
                    n_idx = PART * ghi
                    nc.gpsimd.dma_gather(
                        m[:, :ghi, :], src_hi,
                        idx_tile[:, o_hi:o_hi + n_idx // 16],
                        n_idx, n_idx, C_FEAT,
                        single_packet=False)
                    yhi = ytp.tile([PART, nb * C_FEAT], f32, tag="yhi")
                    nc.vector.tensor_reduce(
                        yhi[:],
                        m[:, :ghi, :].rearrange("p (b l) f -> p b f l", l=lhi),
                        AX.X, OP.add)
                if ylo is not None and yhi is not None:
                    nc.vector.tensor_add(ylo[:], ylo[:], yhi[:])
                    out_cb(ylo, nb, b0)
                elif ylo is not None:
                    out_cb(ylo, nb, b0)
                elif yhi is not None:
                    out_cb(yhi, nb, b0)
                else:
                    y0 = ytp.tile([PART, nb * C_FEAT], f32, tag="ylo")
                    nc.vector.memset(y0[:], 0.0)
                    out_cb(y0, nb, b0)

            # ---------------- recurrence: zhat_k for k = 1..K-1 ------------
            for k in range(1, K):
                if k == 1:
                    src = z0_full
                    src_lo = src.ap()[0:n_lo_tok, :]
                    src_hi = (src.ap()[LO_LIM:N_tok, :]
                              if n_hi_tok > 0 else None)
                else:
                    zf = z_fulls[k - 2]
                    src_lo = zf[0:n_lo_tok, :]
                    src_hi = (zf[LO_LIM:N_tok, :]
                              if n_hi_tok > 0 else None)

                def rec_out(y, nb, b0, k=k):
                    cols = slice(b0 * C_FEAT, (b0 + nb) * C_FEAT)
                    zh = ytp.tile([PART, nb * C_FEAT], f32, tag="zhat")
                    nc.vector.tensor_mul(zh[:], y[:], g_t[:, cols])
                    # stilde += alpha_{k+1} * zhat_k
                    nc.vector.scalar_tensor_tensor(
                        st_t[:, cols], zh[:], float(p.alpha[k + 1]),
                        st_t[:, cols], OP.mult, OP.add)
                    if k < K - 1:
                        nc.sync.dma_start(
                            z_shard[b0 * PART:(b0 + nb) * PART, :]
                            .rearrange("(b q) f -> q b f", q=PART),
                            zh[:].rearrange("p (b f) -> p b f", f=C_FEAT))

                for ci, (b0, nb) in enumerate(p.chunks_rec):
                    spmm_chunk(ci, b0, nb, p.L_lo_rec[ci], p.L_hi_rec[ci],
                               p.offs_rec, idx_t, src_lo, src_hi, rec_out)

                if k < K - 1:
                    nc.gpsimd.collective_compute(
                        "AllGather", mybir.AluOpType.bypass,
                        replica_groups=rg,
                        ins=[z_shard[:].opt()],
                        outs=[z_fulls[k - 1][:].opt()])

            # ---------------- share stilde ---------------------------------
            if getattr(p, "skip_final", False):
                nc.sync.dma_start(
                    out_d.ap()[0:TPC, :].rearrange("(b q) f -> q b f", q=PART),
                    st_t[:].rearrange("p (b f) -> p b f", f=C_FEAT))
                return
            nc.sync.dma_start(
                st_shard[:].rearrange("(b q) f -> q b f", q=PART),
                st_t[:].rearrange("p (b f) -> p b f", f=C_FEAT))
            nc.gpsimd.collective_compute(
                "AllGather", mybir.AluOpType.bypass, replica_groups=rg,
                ins=[st_shard[:].opt()], outs=[st_full[:].opt()])

        # ---------------- final pass: out = resid + dis * (Cnt @ stilde) ---
        with (
            tc.tile_pool(name="fconst", bufs=1) as fconst,
            tc.tile_pool(name="fmsg", bufs=2) as msgp,
            tc.tile_pool(name="fytmp", bufs=3) as ytp,
        ):
            idxf_t = fconst.tile([PART, p.cols_fin], i16, tag="idx_fin")
            g2_t = fconst.tile([PART, B_f * C_FEAT], f32, tag="g2_bcast")
            nc.sync.dma_start(idxf_t[:], idxf_d[:])
            nc.sync.dma_start(g2_t[:], g2b_d[:])
            src_lo = st_full[0:n_lo_tok, :]
            src_hi = st_full[LO_LIM:N_tok, :] if n_hi_tok > 0 else None

            def fin_out(y, nb, b0):
                cols = slice(b0 * C_FEAT, (b0 + nb) * C_FEAT)
                rt = ytp.tile([PART, nb * C_FEAT], f32, tag="resid")
                nc.sync.dma_start(
                    rt[:].rearrange("p (b f) -> p b f", f=C_FEAT),
                    resid_d.ap()[b0 * PART:(b0 + nb) * PART, :]
                    .rearrange("(b q) f -> q b f", q=PART))
                ot = ytp.tile([PART, nb * C_FEAT], f32, tag="outt")
                nc.vector.tensor_mul(ot[:], y[:], g2_t[:, cols])
                nc.vector.tensor_add(ot[:], ot[:], rt[:])
                nc.sync.dma_start(
                    out_d.ap()[b0 * PART:(b0 + nb) * PART, :]
                    .rearrange("(b q) f -> q b f", q=PART),
                    ot[:].rearrange("p (b f) -> p b f", f=C_FEAT))

            def spmm_chunk_f(ci, b0, nb, llo, lhi):
                o_lo, o_hi = p.offs_fin[ci]
                ylo = yhi = None
                if llo > 0:
                    glo = nb * llo
                    m = msgp.tile([PART, max_glo_f, C_FEAT], f32, tag="msg_lo")
                    n_idx = PART * glo
                    nc.gpsimd.dma_gather(
                        m[:, :glo, :], src_lo,
                        idxf_t[:, o_lo:o_lo + n_idx // 16],
                        n_idx, n_idx, C_FEAT,
                        single_packet=False)
                    ylo = ytp.tile([PART, nb * C_FEAT], f32, tag="ylo")
                    nc.vector.tensor_reduce(
                        ylo[:],
                        m[:, :glo, :].rearrange("p (b l) f -> p b f l", l=llo),
                        AX.X, OP.add)
                if lhi > 0:
                    ghi = nb * lhi
                    m = msgp.tile([PART, max_ghi_f, C_FEAT], f32, tag="msg_hi")
                    n_idx = PART * ghi
                    nc.gpsimd.dma_gather(
                        m[:, :ghi, :], src_hi,
                        idxf_t[:, o_hi:o_hi + n_idx // 16],
                        n_idx, n_idx, C_FEAT,
                        single_packet=False)
                    yhi = ytp.tile([PART, nb * C_FEAT], f32, tag="yhi")
                    nc.vector.tensor_reduce(
                        yhi[:],
                        m[:, :ghi, :].rearrange("p (b l) f -> p b f l", l=lhi),
                        AX.X, OP.add)
                if ylo is not None and yhi is not None:
                    nc.vector.tensor_add(ylo[:], ylo[:], yhi[:])
                    fin_out(ylo, nb, b0)
                elif ylo is not None:
                    fin_out(ylo, nb, b0)
                elif yhi is not None:
                    fin_out(yhi, nb, b0)
                else:
                    y0 = ytp.tile([PART, nb * C_FEAT], f32, tag="ylo")
                    nc.vector.memset(y0[:], 0.0)
                    fin_out(y0, nb, b0)

            for ci, (b0, nb) in enumerate(p.chunks_fin):
                spmm_chunk_f(ci, b0, nb, p.L_lo_fin[ci], p.L_hi_fin[ci])


def _in_maps(p):
    maps = []
    for c in range(N_CORES):
        maps.append({
            "z0_full": p.Z0,
            "z0_shard": np.ascontiguousarray(p.Z0[c * p.TPC:(c + 1) * p.TPC]),
            "idx_rec": np.ascontiguousarray(p.idx_rec[c]),
            "idx_fin": np.ascontiguousarray(p.idx_fin[c]),
            "g_bcast": np.ascontiguousarray(p.g_bcast[c]),
            "g2_bcast": np.ascontiguousarray(p.g2_bcast[c]),
            "resid_perm": np.ascontiguousarray(p.resid_perm[c]),
        })
    return maps


def _assemble(p, results):
    out = np.zeros((p.N, C_FEAT), np.float32)
    for c in range(N_CORES):
        sh = results[c]["out_shard"]
        v = p.slot_node_fin[c] >= 0
        out[p.slot_node_fin[c][v]] = sh[v]
    return out


def _make_nc(p):
    from concourse import bacc, tile
    nc = bacc.Bacc("TRN2", target_bir_lowering=False, debug=False,
                   num_devices=N_CORES)
    with tile.TileContext(nc) as tc:
        _build_kernel(nc, tc, p)
    nc.compile()
    return nc


def kernel(residuals, edge_index, train_mask, num_nodes, _K=16,
           _executor="hw", _trace=False, _skip_final=False):
    p = _plan(residuals, edge_index, train_mask, int(num_nodes), K=_K)
    p.skip_final = _skip_final
    nc = _make_nc(p)
    in_maps = _in_maps(p)

    if _executor == "sim":
        from concourse import bass_interp
        sim = bass_interp.MultiCoreSim(nc, N_CORES)
        for c in range(N_CORES):
            for k, v in in_maps[c].items():
                sim.cores[c].tensor(k)[:] = v
        sim.simulate(check_with_hw=False)
        results = [{"out_shard": np.array(sim.cores[c].mem_tensor("out_shard"))}
                   for c in range(N_CORES)]
        return _assemble(p, results)

    from concourse import bass_utils
    r = bass_utils.run_bass_kernel_spmd(
        nc, in_maps, core_ids=list(range(N_CORES)), trace=_trace)
    kernel.last_exec_time_ns = r.exec_time_ns
    return _assemble(p, r.results)

